# revision 1
# baseline (speedup 1.0000x reference)
"""LIF neuron (STBP) forward kernel for Trainium2, 8-core data parallel.

Reference semantics (per element, scan over T):
    v = v * 0.9 + x_t
    s = (v >= 1.0)
    v = v - s * 1.0

Sharding: batch dim 32 -> 8 cores x 4. The recurrence is elementwise per
neuron, so no cross-core communication. Per core, all 4 local batches for
one timestep are fused into a single [128, 2048] SBUF tile (batch-major in
the free dim); the T loop keeps the membrane state v in SBUF and streams
x in / spikes out. Three VectorE ops per timestep:
    u = (v * beta) + x_t        scalar_tensor_tensor, 1x mode
    s = (u >= 1.0)              tensor_scalar is_ge,   2x mode
    v = u - s                   tensor_tensor sub,     1x mode
t=0 skips the first op (v0 = 0 so u = x_0) and computes per-batch so the
first spike tiles reach the store queue early; t=T-1 is also split
per-batch so the tail drains ASAP. DMA issue alternates between the two
HWDGE engines (sync, scalar) to halve per-engine dispatch serialization
and balance the two queue directions.
"""

from contextlib import ExitStack

import numpy as np

import concourse.bacc as bacc
import concourse.mybir as mybir
import concourse.tile as tile
from concourse.bass_utils import run_bass_kernel_spmd

N_CORES = 8
B, T, C, H, W = 32, 16, 64, 32, 32
B_LOC = B // N_CORES  # 4 batches per core
P = 128               # SBUF partitions
F = (C * H * W) // P  # 512 free elements per partition per batch
FB = B_LOC * F        # 2048 free elements in a fused all-batch tile
BETA = 0.9
V_TH = 1.0

_CACHE = {}


def _build(repeat: int = 1, loop_n: int = 1):
    """Build + compile the per-core Bass program (identical on all cores).

    repeat > 1 unrolls the whole (idempotent) kernel body that many times
    inside one NEFF; loop_n > 1 additionally wraps it in a hardware loop.
    Both are used only for wall-clock timing of the device step.
    """
    nc = bacc.Bacc(
        "TRN2", target_bir_lowering=False, debug=False, num_devices=N_CORES
    )
    x = nc.dram_tensor(
        "x", [B_LOC, T, P, F], mybir.dt.float32, kind="ExternalInput"
    ).ap()
    s_out = nc.dram_tensor(
        "s", [B_LOC, T, P, F], mybir.dt.float32, kind="ExternalOutput"
    ).ap()

    with tile.TileContext(nc) as tc:
        def emit_body():
            _emit(nc, tc, x, s_out, repeat)

        if loop_n > 1:
            with tc.For_i(
                0, loop_n, 1,
                hint_engines=(
                    mybir.EngineType.SP,
                    mybir.EngineType.Activation,
                    mybir.EngineType.DVE,
                ),
            ):
                emit_body()
        else:
            emit_body()

    nc.compile()
    return nc


def _emit(nc, tc, x, s_out, repeat):
    with ExitStack() as ctx:
        xp = ctx.enter_context(tc.tile_pool(name="xp", bufs=6))
        up = ctx.enter_context(tc.tile_pool(name="up", bufs=2))
        sp = ctx.enter_context(tc.tile_pool(name="sp", bufs=4))
        vp = ctx.enter_context(tc.tile_pool(name="vp", bufs=2))

        # Alternate HWDGE issuing engine per (t, b) so each engine carries
        # half the inputs and half the outputs.
        def in_eng(t, b):
            return nc.sync if (t * B_LOC + b) % 2 == 0 else nc.scalar

        def out_eng(t, b):
            return nc.scalar if (t * B_LOC + b) % 2 == 0 else nc.sync

        v = None
        for t in [t for _ in range(repeat) for t in range(T)]:
            xt = xp.tile([P, FB], mybir.dt.float32)
            for b in range(B_LOC):
                in_eng(t, b).dma_start(xt[:, b * F:(b + 1) * F], x[b, t])

            st = sp.tile([P, FB], mybir.dt.float32)

            if t == 0:
                # v0 = 0 -> u = x0; compute per-batch so spikes for batch b
                # are ready as soon as its input lands.
                vn = vp.tile([P, FB], mybir.dt.float32)
                for b in range(B_LOC):
                    sl = slice(b * F, (b + 1) * F)
                    nc.vector.tensor_scalar(
                        st[:, sl], xt[:, sl], V_TH, None, mybir.AluOpType.is_ge
                    )
                    out_eng(t, b).dma_start(s_out[b, t], st[:, sl])
                    nc.vector.tensor_sub(vn[:, sl], xt[:, sl], st[:, sl])
                v = vn
                continue

            u = up.tile([P, FB], mybir.dt.float32)
            if t < T - 1:
                nc.vector.scalar_tensor_tensor(
                    u[:], v[:], BETA, xt[:],
                    mybir.AluOpType.mult, mybir.AluOpType.add,
                )
                nc.vector.tensor_scalar(
                    st[:], u[:], V_TH, None, mybir.AluOpType.is_ge
                )
                for b in range(B_LOC):
                    out_eng(t, b).dma_start(s_out[b, t], st[:, b * F:(b + 1) * F])
                vn = vp.tile([P, FB], mybir.dt.float32)
                nc.vector.tensor_sub(vn[:], u[:], st[:])
                v = vn
            else:
                # Last step: no v update needed; split per-batch so the
                # final stores start draining before all compute finishes.
                for b in range(B_LOC):
                    sl = slice(b * F, (b + 1) * F)
                    nc.vector.scalar_tensor_tensor(
                        u[:, sl], v[:, sl], BETA, xt[:, sl],
                        mybir.AluOpType.mult, mybir.AluOpType.add,
                    )
                    nc.vector.tensor_scalar(
                        st[:, sl], u[:, sl], V_TH, None, mybir.AluOpType.is_ge
                    )
                    out_eng(t, b).dma_start(s_out[b, t], st[:, sl])


def _get_nc(repeat: int = 1, loop_n: int = 1):
    key = f"nc{repeat}_{loop_n}"
    if key not in _CACHE:
        _CACHE[key] = _build(repeat, loop_n)
    return _CACHE[key]


def _run(x_seq: np.ndarray, trace: bool = False, repeat: int = 1):
    """Shard, execute on 8 cores, gather. Returns (output, BassKernelResults)."""
    nc = _get_nc(repeat)
    x_seq = np.ascontiguousarray(x_seq, dtype=np.float32)
    in_maps = [
        {"x": x_seq[i * B_LOC:(i + 1) * B_LOC].reshape(B_LOC, T, P, F)}
        for i in range(N_CORES)
    ]
    res = run_bass_kernel_spmd(
        nc, in_maps, core_ids=list(range(N_CORES)), trace=trace
    )
    out = np.concatenate(
        [r["s"].reshape(B_LOC, T, C, H, W) for r in res.results], axis=0
    )
    return out, res


def kernel(x_seq: np.ndarray) -> np.ndarray:
    out, _ = _run(x_seq, trace=False)
    return out



# revision 3
# speedup vs baseline: 1.0055x; 1.0055x over previous
"""LIF neuron (STBP) forward kernel for Trainium2, 8-core data parallel.

Reference semantics (per element, scan over T):
    u = v * 0.9 + x_t
    s = (u >= 1.0)
    v = u - s * 1.0

Sharding: batch dim 32 -> 8 cores x 4. Per core each timestep is a
[128, 2048] fp32 tile (free dim = 4 local batches x 512).

v2 design (vs the all-VectorE baseline):
  - The threshold compare runs on the otherwise-idle Activation engine
    as s = Sign(u - 1) with uint8 output: Sign yields -1/0/+1 and the
    float->u8 conversion saturates to exactly {0, 1} = (u >= 1), checked
    on hardware. Spikes therefore leave the chip as 1 byte/element
    (4x less write traffic); the host widens them back to fp32.
  - The membrane update splits across engines:
        VectorE : u = (v * beta) + x          (scalar_tensor_tensor, full)
        VectorE : v = (s * -1) + u            (first A2 of each 1024-chunk)
        GPSIMD  : v = u - s                   (rest; tensor_tensor, u8 rhs)
  - Each step is processed in CH free-dim chunks so the three engines
    pipeline within a step instead of serializing on the u -> s -> v
    dependency chain.
  - HBM layout is host-side retiled so input arrives as one
    [128 x 16 KiB] DMA per 2 steps and spikes leave as one
    [128 x 8 KiB] u8 DMA per 4 steps (>= 1 MiB transfers).
  - t = 0 skips the u-update (v0 = 0 so u = x0); t = T-1 skips the
    v-update (v unused afterwards).
"""

from contextlib import ExitStack

import numpy as np

import concourse.bacc as bacc
import concourse.mybir as mybir
import concourse.tile as tile
from concourse.bass_utils import run_bass_kernel_spmd

N_CORES = 8
B, T, C, H, W = 32, 16, 64, 32, 32
B_LOC = B // N_CORES          # 4 batches per core
P = 128                       # SBUF partitions
F = (C * H * W) // P          # 512 free elements per partition per batch
FB = B_LOC * F                # 2048 free elements per timestep tile
BETA = 0.9

CH = 2                        # pipeline chunks per step
CK = FB // CH                 # chunk size (1024)
A2 = 384                      # VectorE share of the v-update per chunk
XPAIR = 2                     # timesteps per input DMA
SQUAD = 4                     # timesteps per output DMA

_CACHE = {}


def _build():
    nc = bacc.Bacc(
        "TRN2", target_bir_lowering=False, debug=False, num_devices=N_CORES
    )
    x = nc.dram_tensor(
        "x", [T // XPAIR, P, XPAIR * FB], mybir.dt.float32, kind="ExternalInput"
    ).ap()
    s_out = nc.dram_tensor(
        "s", [T // SQUAD, P, SQUAD * FB], mybir.dt.uint8, kind="ExternalOutput"
    ).ap()

    with tile.TileContext(nc) as tc:
        _emit(nc, tc, x, s_out)

    nc.compile()
    return nc


def _emit(nc, tc, x, s_out):
    with ExitStack() as ctx:
        cpool = ctx.enter_context(tc.tile_pool(name="cp", bufs=1))
        xp = ctx.enter_context(tc.tile_pool(name="xp", bufs=3))
        up = ctx.enter_context(tc.tile_pool(name="up", bufs=2))
        sp = ctx.enter_context(tc.tile_pool(name="sp", bufs=2))
        vp = ctx.enter_context(tc.tile_pool(name="vp", bufs=2))

        bias_m1 = cpool.tile([P, 1], mybir.dt.float32)
        nc.gpsimd.memset(bias_m1, -1.0)

        xt = None
        st = None
        v = None
        for t in range(T):
            if t % XPAIR == 0:
                xt = xp.tile([P, XPAIR * FB], mybir.dt.float32)
                nc.sync.dma_start(xt, x[t // XPAIR])
            if t % SQUAD == 0:
                st = sp.tile([P, SQUAD * FB], mybir.dt.uint8)
            xoff = (t % XPAIR) * FB
            soff = (t % SQUAD) * FB

            u = (
                up.tile([P, FB], mybir.dt.float32, name="u") if t > 0 else None
            )
            vn = (
                vp.tile([P, FB], mybir.dt.float32, name="vn")
                if t < T - 1
                else None
            )
            for c in range(CH):
                lo = c * CK
                hi = lo + CK
                if t == 0:
                    # v0 = 0 -> u = x0: read spikes straight off the x tile.
                    uc = xt[:, xoff + lo:xoff + hi]
                else:
                    uc = u[:, lo:hi]
                    nc.vector.scalar_tensor_tensor(
                        uc, v[:, lo:hi], BETA, xt[:, xoff + lo:xoff + hi],
                        mybir.AluOpType.mult, mybir.AluOpType.add,
                    )
                sc = st[:, soff + lo:soff + hi]
                nc.scalar.activation(
                    sc, uc, mybir.ActivationFunctionType.Sign,
                    bias=bias_m1, scale=1.0,
                )
                if t < T - 1:
                    nc.vector.scalar_tensor_tensor(
                        vn[:, lo:lo + A2],
                        st[:, soff + lo:soff + lo + A2], -1.0,
                        uc[:, :A2] if t == 0 else u[:, lo:lo + A2],
                        mybir.AluOpType.mult, mybir.AluOpType.add,
                    )
                    nc.gpsimd.tensor_tensor(
                        vn[:, lo + A2:hi],
                        uc[:, A2:] if t == 0 else u[:, lo + A2:hi],
                        st[:, soff + lo + A2:soff + hi],
                        mybir.AluOpType.subtract,
                    )
            v = vn
            if t % SQUAD == SQUAD - 1:
                nc.scalar.dma_start(s_out[t // SQUAD], st)


def _get_nc():
    if "nc" not in _CACHE:
        _CACHE["nc"] = _build()
    return _CACHE["nc"]


def _shard_inputs(x_seq: np.ndarray):
    """[B, T, C, H, W] f32 -> per-core [T//XPAIR, P, XPAIR*FB] device layout."""
    x_seq = np.ascontiguousarray(x_seq, dtype=np.float32)
    maps = []
    for i in range(N_CORES):
        xc = x_seq[i * B_LOC:(i + 1) * B_LOC].reshape(B_LOC, T, P, F)
        # [b, t, p, f] -> [tpair, p, (j, b, f)]
        xc = xc.transpose(1, 2, 0, 3)                        # [t, p, b, f]
        xc = xc.reshape(T // XPAIR, XPAIR, P, B_LOC * F)     # [tp, j, p, bf]
        xc = np.ascontiguousarray(xc.transpose(0, 2, 1, 3))  # [tp, p, j, bf]
        maps.append({"x": xc.reshape(T // XPAIR, P, XPAIR * FB)})
    return maps


def _unshard_output(results) -> np.ndarray:
    outs = []
    for r in results:
        sd = np.asarray(r["s"]).reshape(T // SQUAD, P, SQUAD, B_LOC, F)
        sd = sd.transpose(3, 0, 2, 1, 4)                     # [b, g, k, p, f]
        sd = sd.reshape(B_LOC, T, C, H, W)
        outs.append(sd)
    return np.concatenate(outs, axis=0).astype(np.float32)


def _run(x_seq: np.ndarray, trace: bool = False):
    nc = _get_nc()
    in_maps = _shard_inputs(x_seq)
    res = run_bass_kernel_spmd(
        nc, in_maps, core_ids=list(range(N_CORES)), trace=trace
    )
    return _unshard_output(res.results), res


def kernel(x_seq: np.ndarray) -> np.ndarray:
    out, _ = _run(x_seq, trace=False)
    return out


# revision 4
# speedup vs baseline: 1.1654x; 1.1590x over previous
"""LIF neuron (STBP) forward kernel for Trainium2, 8-core data parallel.

Reference semantics (per element, scan over T):
    u = v * 0.9 + x_t
    s = (u >= 1.0)
    v = u - s * 1.0

Sharding: batch dim 32 -> 8 cores x 4. Per core each timestep is a
[128, 2048] fp32 tile (free dim = 4 local batches x 512).

v2 design (vs the all-VectorE baseline):
  - The threshold compare runs on the otherwise-idle Activation engine
    as s = Sign(u - 1) with uint8 output: Sign yields -1/0/+1 and the
    float->u8 conversion saturates to exactly {0, 1} = (u >= 1), checked
    on hardware. Spikes therefore leave the chip as 1 byte/element
    (4x less write traffic); the host widens them back to fp32.
  - The membrane update splits across engines:
        VectorE : u = (v * beta) + x          (scalar_tensor_tensor, full)
        VectorE : v = (s * -1) + u            (first A2 of each 1024-chunk)
        GPSIMD  : v = u - s                   (rest; tensor_tensor, u8 rhs)
  - Each step is processed in CH free-dim chunks so the three engines
    pipeline within a step instead of serializing on the u -> s -> v
    dependency chain.
  - HBM layout is host-side retiled so input arrives as one
    [128 x 16 KiB] DMA per 2 steps and spikes leave as one
    [128 x 8 KiB] u8 DMA per 4 steps (>= 1 MiB transfers).
  - t = 0 skips the u-update (v0 = 0 so u = x0); t = T-1 skips the
    v-update (v unused afterwards).
"""

from contextlib import ExitStack

import numpy as np

import concourse.bacc as bacc
import concourse.mybir as mybir
import concourse.tile as tile
from concourse.bass_utils import run_bass_kernel_spmd

N_CORES = 8
B, T, C, H, W = 32, 16, 64, 32, 32
B_LOC = B // N_CORES          # 4 batches per core
P = 128                       # SBUF partitions
F = (C * H * W) // P          # 512 free elements per partition per batch
FB = B_LOC * F                # 2048 free elements per timestep tile
BETA = 0.9

CH = 2                        # pipeline chunks per step
CK = FB // CH                 # chunk size (1024)
A2 = 384                      # VectorE share of the v-update per chunk
XPAIR = 2                     # timesteps per input DMA
SQUAD = 4                     # timesteps per output DMA

_CACHE = {}


def _build():
    nc = bacc.Bacc(
        "TRN2", target_bir_lowering=False, debug=False, num_devices=N_CORES
    )
    x = nc.dram_tensor(
        "x", [T // XPAIR, P, XPAIR * FB], mybir.dt.float32, kind="ExternalInput"
    ).ap()
    s_out = nc.dram_tensor(
        "s", [T // SQUAD, P, SQUAD * FB], mybir.dt.uint8, kind="ExternalOutput"
    ).ap()

    with tile.TileContext(nc) as tc:
        _emit(nc, tc, x, s_out)

    nc.compile()
    return nc


def _emit(nc, tc, x, s_out):
    with ExitStack() as ctx:
        cpool = ctx.enter_context(tc.tile_pool(name="cp", bufs=1))
        xp = ctx.enter_context(tc.tile_pool(name="xp", bufs=3))
        up = ctx.enter_context(tc.tile_pool(name="up", bufs=2))
        sp = ctx.enter_context(tc.tile_pool(name="sp", bufs=2))
        vp = ctx.enter_context(tc.tile_pool(name="vp", bufs=2))

        bias_m1 = cpool.tile([P, 1], mybir.dt.float32)
        nc.gpsimd.memset(bias_m1, -1.0)

        xt = None
        st = None
        v = None
        for t in range(T):
            if t % XPAIR == 0:
                xt = xp.tile([P, XPAIR * FB], mybir.dt.float32)
                nc.sync.dma_start(xt, x[t // XPAIR])
            if t % SQUAD == 0:
                st = sp.tile([P, SQUAD * FB], mybir.dt.uint8)
            xoff = (t % XPAIR) * FB
            soff = (t % SQUAD) * FB

            u = (
                up.tile([P, FB], mybir.dt.float32, name="u") if t > 0 else None
            )
            vn = (
                vp.tile([P, FB], mybir.dt.float32, name="vn")
                if t < T - 1
                else None
            )
            for c in range(CH):
                lo = c * CK
                hi = lo + CK
                if t == 0:
                    # v0 = 0 -> u = x0: read spikes straight off the x tile.
                    uc = xt[:, xoff + lo:xoff + hi]
                else:
                    uc = u[:, lo:hi]
                    nc.vector.scalar_tensor_tensor(
                        uc, v[:, lo:hi], BETA, xt[:, xoff + lo:xoff + hi],
                        mybir.AluOpType.mult, mybir.AluOpType.add,
                    )
                sc = st[:, soff + lo:soff + hi]
                nc.scalar.activation(
                    sc, uc, mybir.ActivationFunctionType.Sign,
                    bias=bias_m1, scale=1.0,
                )
                if t < T - 1:
                    nc.vector.scalar_tensor_tensor(
                        vn[:, lo:hi], sc, -1.0, uc,
                        mybir.AluOpType.mult, mybir.AluOpType.add,
                    )
            v = vn
            if t % SQUAD == SQUAD - 1:
                nc.scalar.dma_start(s_out[t // SQUAD], st)


def _get_nc():
    if "nc" not in _CACHE:
        _CACHE["nc"] = _build()
    return _CACHE["nc"]


def _shard_inputs(x_seq: np.ndarray):
    """[B, T, C, H, W] f32 -> per-core [T//XPAIR, P, XPAIR*FB] device layout."""
    x_seq = np.ascontiguousarray(x_seq, dtype=np.float32)
    maps = []
    for i in range(N_CORES):
        xc = x_seq[i * B_LOC:(i + 1) * B_LOC].reshape(B_LOC, T, P, F)
        # [b, t, p, f] -> [tpair, p, (j, b, f)]
        xc = xc.transpose(1, 2, 0, 3)                        # [t, p, b, f]
        xc = xc.reshape(T // XPAIR, XPAIR, P, B_LOC * F)     # [tp, j, p, bf]
        xc = np.ascontiguousarray(xc.transpose(0, 2, 1, 3))  # [tp, p, j, bf]
        maps.append({"x": xc.reshape(T // XPAIR, P, XPAIR * FB)})
    return maps


def _unshard_output(results) -> np.ndarray:
    outs = []
    for r in results:
        sd = np.asarray(r["s"]).reshape(T // SQUAD, P, SQUAD, B_LOC, F)
        sd = sd.transpose(3, 0, 2, 1, 4)                     # [b, g, k, p, f]
        sd = sd.reshape(B_LOC, T, C, H, W)
        outs.append(sd)
    return np.concatenate(outs, axis=0).astype(np.float32)


def _run(x_seq: np.ndarray, trace: bool = False):
    nc = _get_nc()
    in_maps = _shard_inputs(x_seq)
    res = run_bass_kernel_spmd(
        nc, in_maps, core_ids=list(range(N_CORES)), trace=trace
    )
    return _unshard_output(res.results), res


def kernel(x_seq: np.ndarray) -> np.ndarray:
    out, _ = _run(x_seq, trace=False)
    return out


# revision 6
# speedup vs baseline: 1.2180x; 1.0451x over previous
"""LIF neuron (STBP) forward kernel for Trainium2, 8-core data parallel.

Reference semantics (per element, scan over T):
    u = v * 0.9 + x_t
    s = (u >= 1.0)
    v = u - s * 1.0

Sharding: batch dim 32 -> 8 cores x 4. Per core each timestep is a
[128, 2048] fp32 tile (free dim = 4 local batches x 512).

v2 design (vs the all-VectorE baseline):
  - The threshold compare runs on the otherwise-idle Activation engine
    as s = Sign(u - 1) with uint8 output: Sign yields -1/0/+1 and the
    float->u8 conversion saturates to exactly {0, 1} = (u >= 1), checked
    on hardware. Spikes therefore leave the chip as 1 byte/element
    (4x less write traffic); the host widens them back to fp32.
  - The membrane update splits across engines:
        VectorE : u = (v * beta) + x          (scalar_tensor_tensor, full)
        VectorE : v = (s * -1) + u            (first A2 of each 1024-chunk)
        GPSIMD  : v = u - s                   (rest; tensor_tensor, u8 rhs)
  - Each step is processed in CH free-dim chunks so the three engines
    pipeline within a step instead of serializing on the u -> s -> v
    dependency chain.
  - HBM layout is host-side retiled so input arrives as one
    [128 x 16 KiB] DMA per 2 steps and spikes leave as one
    [128 x 8 KiB] u8 DMA per 4 steps (>= 1 MiB transfers).
  - t = 0 skips the u-update (v0 = 0 so u = x0); t = T-1 skips the
    v-update (v unused afterwards).
"""

from contextlib import ExitStack

import numpy as np

import concourse.bacc as bacc
import concourse.mybir as mybir
import concourse.tile as tile
from concourse.bass_utils import run_bass_kernel_spmd

N_CORES = 8
B, T, C, H, W = 32, 16, 64, 32, 32
B_LOC = B // N_CORES          # 4 batches per core
P = 128                       # SBUF partitions
F = (C * H * W) // P          # 512 free elements per partition per batch
FB = B_LOC * F                # 2048 free elements per timestep tile
BETA = 0.9

CH = 2                        # pipeline chunks per step
CK = FB // CH                 # chunk size (1024)
A2 = 384                      # VectorE share of the v-update per chunk
XPAIR = 2                     # timesteps per input DMA
SQUAD = 4                     # timesteps per output DMA

_CACHE = {}


def _build():
    nc = bacc.Bacc(
        "TRN2", target_bir_lowering=False, debug=False, num_devices=N_CORES
    )
    x = nc.dram_tensor(
        "x", [T // XPAIR, P, XPAIR * FB], mybir.dt.float32, kind="ExternalInput"
    ).ap()
    s_out = nc.dram_tensor(
        "s", [T // SQUAD, P, SQUAD * FB], mybir.dt.uint8, kind="ExternalOutput"
    ).ap()

    with tile.TileContext(nc) as tc:
        _emit(nc, tc, x, s_out)

    nc.compile()
    return nc


def _emit(nc, tc, x, s_out):
    with ExitStack() as ctx:
        cpool = ctx.enter_context(tc.tile_pool(name="cp", bufs=1))
        xp = ctx.enter_context(tc.tile_pool(name="xp", bufs=3))
        up = ctx.enter_context(tc.tile_pool(name="up", bufs=2))
        sp = ctx.enter_context(tc.tile_pool(name="sp", bufs=2))
        vp = ctx.enter_context(tc.tile_pool(name="vp", bufs=2))

        bias_m1 = cpool.tile([P, 1], mybir.dt.float32)
        nc.gpsimd.memset(bias_m1, -1.0)
        # Dummy 1-element Sign so the ACT table load (~2.7us) overlaps the
        # first input DMA instead of serializing after it.
        warm = cpool.tile([P, 1], mybir.dt.uint8)
        nc.scalar.activation(
            warm, bias_m1, mybir.ActivationFunctionType.Sign,
            bias=bias_m1, scale=1.0,
        )

        xt = None
        st = None
        v = None
        for t in range(T):
            if t % XPAIR == 0:
                xt = xp.tile([P, XPAIR * FB], mybir.dt.float32)
                if t == 0:
                    # Split the first load so Sign(t0, chunk0) can start
                    # after ~0.5 MiB instead of the full 2 MiB.
                    nc.sync.dma_start(xt[:, :CK], x[0][:, :CK])
                    nc.sync.dma_start(xt[:, CK:FB], x[0][:, CK:FB])
                    nc.sync.dma_start(xt[:, FB:], x[0][:, FB:])
                else:
                    nc.sync.dma_start(xt, x[t // XPAIR])
            if t % SQUAD == 0:
                st = sp.tile([P, SQUAD * FB], mybir.dt.uint8)
            xoff = (t % XPAIR) * FB
            soff = (t % SQUAD) * FB

            u = (
                up.tile([P, FB], mybir.dt.float32, name="u") if t > 0 else None
            )
            vn = (
                vp.tile([P, FB], mybir.dt.float32, name="vn")
                if t < T - 1
                else None
            )
            for c in range(CH):
                lo = c * CK
                hi = lo + CK
                if t == 0:
                    # v0 = 0 -> u = x0: read spikes straight off the x tile.
                    uc = xt[:, xoff + lo:xoff + hi]
                else:
                    uc = u[:, lo:hi]
                    nc.vector.scalar_tensor_tensor(
                        uc, v[:, lo:hi], BETA, xt[:, xoff + lo:xoff + hi],
                        mybir.AluOpType.mult, mybir.AluOpType.add,
                    )
                sc = st[:, soff + lo:soff + hi]
                nc.scalar.activation(
                    sc, uc, mybir.ActivationFunctionType.Sign,
                    bias=bias_m1, scale=1.0,
                )
                if t < T - 1:
                    nc.vector.scalar_tensor_tensor(
                        vn[:, lo:hi], sc, -1.0, uc,
                        mybir.AluOpType.mult, mybir.AluOpType.add,
                    )
            v = vn
            if t >= T - SQUAD:
                # Tail: drain each step's spikes as soon as they're ready.
                nc.scalar.dma_start(
                    s_out[t // SQUAD][:, soff:soff + FB], st[:, soff:soff + FB]
                )
            elif t % SQUAD == SQUAD - 1:
                nc.scalar.dma_start(s_out[t // SQUAD], st)


def _get_nc():
    if "nc" not in _CACHE:
        _CACHE["nc"] = _build()
    return _CACHE["nc"]


def _shard_inputs(x_seq: np.ndarray):
    """[B, T, C, H, W] f32 -> per-core [T//XPAIR, P, XPAIR*FB] device layout."""
    x_seq = np.ascontiguousarray(x_seq, dtype=np.float32)
    maps = []
    for i in range(N_CORES):
        xc = x_seq[i * B_LOC:(i + 1) * B_LOC].reshape(B_LOC, T, P, F)
        # [b, t, p, f] -> [tpair, p, (j, b, f)]
        xc = xc.transpose(1, 2, 0, 3)                        # [t, p, b, f]
        xc = xc.reshape(T // XPAIR, XPAIR, P, B_LOC * F)     # [tp, j, p, bf]
        xc = np.ascontiguousarray(xc.transpose(0, 2, 1, 3))  # [tp, p, j, bf]
        maps.append({"x": xc.reshape(T // XPAIR, P, XPAIR * FB)})
    return maps


def _unshard_output(results) -> np.ndarray:
    outs = []
    for r in results:
        sd = np.asarray(r["s"]).reshape(T // SQUAD, P, SQUAD, B_LOC, F)
        sd = sd.transpose(3, 0, 2, 1, 4)                     # [b, g, k, p, f]
        sd = sd.reshape(B_LOC, T, C, H, W)
        outs.append(sd)
    return np.concatenate(outs, axis=0).astype(np.float32)


def _run(x_seq: np.ndarray, trace: bool = False):
    nc = _get_nc()
    in_maps = _shard_inputs(x_seq)
    res = run_bass_kernel_spmd(
        nc, in_maps, core_ids=list(range(N_CORES)), trace=trace
    )
    return _unshard_output(res.results), res


def kernel(x_seq: np.ndarray) -> np.ndarray:
    out, _ = _run(x_seq, trace=False)
    return out


# revision 8
# speedup vs baseline: 1.2460x; 1.0230x over previous
"""LIF neuron (STBP) forward kernel for Trainium2, 8-core data parallel.

Reference semantics (per element, scan over T):
    u = v * 0.9 + x_t
    s = (u >= 1.0)
    v = u - s * 1.0

Sharding: batch dim 32 -> 8 cores x 4. Per core each timestep is a
[128, 2048] fp32 tile (free dim = 4 local batches x 512).

v2 design (vs the all-VectorE baseline):
  - The threshold compare runs on the otherwise-idle Activation engine
    as s = Sign(u - 1) with uint8 output: Sign yields -1/0/+1 and the
    float->u8 conversion saturates to exactly {0, 1} = (u >= 1), checked
    on hardware. Spikes therefore leave the chip as 1 byte/element
    (4x less write traffic); the host widens them back to fp32.
  - The membrane update splits across engines:
        VectorE : u = (v * beta) + x          (scalar_tensor_tensor, full)
        VectorE : v = (s * -1) + u            (first A2 of each 1024-chunk)
        GPSIMD  : v = u - s                   (rest; tensor_tensor, u8 rhs)
  - Each step is processed in CH free-dim chunks so the three engines
    pipeline within a step instead of serializing on the u -> s -> v
    dependency chain.
  - HBM layout is host-side retiled so input arrives as one
    [128 x 16 KiB] DMA per 2 steps and spikes leave as one
    [128 x 8 KiB] u8 DMA per 4 steps (>= 1 MiB transfers).
  - t = 0 skips the u-update (v0 = 0 so u = x0); t = T-1 skips the
    v-update (v unused afterwards).
"""

from contextlib import ExitStack

import numpy as np

import concourse.bacc as bacc
import concourse.mybir as mybir
import concourse.tile as tile
from concourse.bass_utils import run_bass_kernel_spmd

N_CORES = 8
B, T, C, H, W = 32, 16, 64, 32, 32
B_LOC = B // N_CORES          # 4 batches per core
P = 128                       # SBUF partitions
F = (C * H * W) // P          # 512 free elements per partition per batch
FB = B_LOC * F                # 2048 free elements per timestep tile
BETA = 0.9

CH = 2                        # pipeline chunks per step
CK = FB // CH                 # chunk size (1024)
A2 = 384                      # VectorE share of the v-update per chunk
XPAIR = 2                     # timesteps per input DMA
SQUAD = 4                     # timesteps per output DMA

_CACHE = {}


def _build():
    nc = bacc.Bacc(
        "TRN2", target_bir_lowering=False, debug=False, num_devices=N_CORES
    )
    x = nc.dram_tensor(
        "x", [T // XPAIR, P, XPAIR * FB], mybir.dt.float32, kind="ExternalInput"
    ).ap()
    s_out = nc.dram_tensor(
        "s", [T // SQUAD, P, SQUAD * FB], mybir.dt.uint8, kind="ExternalOutput"
    ).ap()

    with tile.TileContext(nc) as tc:
        _emit(nc, tc, x, s_out)

    nc.compile()
    return nc


def _emit(nc, tc, x, s_out):
    with ExitStack() as ctx:
        cpool = ctx.enter_context(tc.tile_pool(name="cp", bufs=1))
        xp = ctx.enter_context(tc.tile_pool(name="xp", bufs=3))
        up = ctx.enter_context(tc.tile_pool(name="up", bufs=2))
        sp = ctx.enter_context(tc.tile_pool(name="sp", bufs=2))
        vp = ctx.enter_context(tc.tile_pool(name="vp", bufs=2))

        bias_m1 = cpool.tile([P, 1], mybir.dt.float32)
        nc.gpsimd.memset(bias_m1, -1.0)
        # Dummy 1-element Sign so the ACT table load (~2.7us) overlaps the
        # first input DMA instead of serializing after it.
        warm = cpool.tile([P, 1], mybir.dt.uint8)
        nc.scalar.activation(
            warm, bias_m1, mybir.ActivationFunctionType.Sign,
            bias=bias_m1, scale=1.0,
        )

        xt = None
        st = None
        v = None
        for t in range(T):
            if t % XPAIR == 0:
                xt = xp.tile([P, XPAIR * FB], mybir.dt.float32)
                if t == 0:
                    # Split the first load so Sign(t0, chunk0) can start
                    # after ~0.5 MiB instead of the full 2 MiB; spread the
                    # triggers over both HWDGE rings so they don't serialize.
                    nc.sync.dma_start(xt[:, :CK], x[0][:, :CK])
                    nc.scalar.dma_start(xt[:, CK:FB], x[0][:, CK:FB])
                    nc.sync.dma_start(xt[:, FB:], x[0][:, FB:])
                else:
                    nc.sync.dma_start(xt, x[t // XPAIR])
            if t % SQUAD == 0:
                st = sp.tile([P, SQUAD * FB], mybir.dt.uint8)
            xoff = (t % XPAIR) * FB
            soff = (t % SQUAD) * FB

            u = (
                up.tile([P, FB], mybir.dt.float32, name="u") if t > 0 else None
            )
            vn = (
                vp.tile([P, FB], mybir.dt.float32, name="vn")
                if t < T - 1
                else None
            )
            for c in range(CH):
                lo = c * CK
                hi = lo + CK
                if t == 0:
                    # v0 = 0 -> u = x0: read spikes straight off the x tile.
                    uc = xt[:, xoff + lo:xoff + hi]
                else:
                    uc = u[:, lo:hi]
                    nc.vector.scalar_tensor_tensor(
                        uc, v[:, lo:hi], BETA, xt[:, xoff + lo:xoff + hi],
                        mybir.AluOpType.mult, mybir.AluOpType.add,
                    )
                sc = st[:, soff + lo:soff + hi]
                nc.scalar.activation(
                    sc, uc, mybir.ActivationFunctionType.Sign,
                    bias=bias_m1, scale=1.0,
                )
                if t < T - 1:
                    nc.vector.scalar_tensor_tensor(
                        vn[:, lo:hi], sc, -1.0, uc,
                        mybir.AluOpType.mult, mybir.AluOpType.add,
                    )
            v = vn
            if t == T - 1:
                # Last step: drain each chunk as soon as its Sign lands.
                for c in range(CH):
                    lo = c * CK
                    nc.scalar.dma_start(
                        s_out[t // SQUAD][:, soff + lo:soff + lo + CK],
                        st[:, soff + lo:soff + lo + CK],
                    )
            elif t >= T - SQUAD:
                # Tail: drain each step's spikes as soon as they're ready.
                nc.scalar.dma_start(
                    s_out[t // SQUAD][:, soff:soff + FB], st[:, soff:soff + FB]
                )
            elif t % SQUAD == SQUAD - 1:
                nc.scalar.dma_start(s_out[t // SQUAD], st)


def _get_nc():
    if "nc" not in _CACHE:
        _CACHE["nc"] = _build()
    return _CACHE["nc"]


def _shard_inputs(x_seq: np.ndarray):
    """[B, T, C, H, W] f32 -> per-core [T//XPAIR, P, XPAIR*FB] device layout."""
    x_seq = np.ascontiguousarray(x_seq, dtype=np.float32)
    maps = []
    for i in range(N_CORES):
        xc = x_seq[i * B_LOC:(i + 1) * B_LOC].reshape(B_LOC, T, P, F)
        # [b, t, p, f] -> [tpair, p, (j, b, f)]
        xc = xc.transpose(1, 2, 0, 3)                        # [t, p, b, f]
        xc = xc.reshape(T // XPAIR, XPAIR, P, B_LOC * F)     # [tp, j, p, bf]
        xc = np.ascontiguousarray(xc.transpose(0, 2, 1, 3))  # [tp, p, j, bf]
        maps.append({"x": xc.reshape(T // XPAIR, P, XPAIR * FB)})
    return maps


def _unshard_output(results) -> np.ndarray:
    outs = []
    for r in results:
        sd = np.asarray(r["s"]).reshape(T // SQUAD, P, SQUAD, B_LOC, F)
        sd = sd.transpose(3, 0, 2, 1, 4)                     # [b, g, k, p, f]
        sd = sd.reshape(B_LOC, T, C, H, W)
        outs.append(sd)
    return np.concatenate(outs, axis=0).astype(np.float32)


def _run(x_seq: np.ndarray, trace: bool = False):
    nc = _get_nc()
    in_maps = _shard_inputs(x_seq)
    res = run_bass_kernel_spmd(
        nc, in_maps, core_ids=list(range(N_CORES)), trace=trace
    )
    return _unshard_output(res.results), res


def kernel(x_seq: np.ndarray) -> np.ndarray:
    out, _ = _run(x_seq, trace=False)
    return out
